# revision 5
# baseline (speedup 1.0000x reference)
"""Two-layer GAT (PyG-style, eval mode) on 8 Trainium2 NeuronCores.

Sharding: edges partitioned by destination-node range (6250 nodes/core).
Each core fully owns the segment-softmax + aggregation for its dst range.

v2 layout (vs baseline):
- ONE dma_gather per chunk (16 groups = 2048 edges) instead of two per
  4-group chunk: layer 1 gathers x rows transposed [feat, edge] only and
  computes h1 = x@W1 per edge group on the PE (512-col matmul); layer 2
  gathers h2e rows direct [edge, feat] only (asrc2 is a packed column).
- Messages m = h1 * p built by DVE reading h1 straight from PSUM
  (512 elems/edge instead of 1024 in the old x-space aggregation).
- S^T transposes batched 8-per-PSUM-bank with one Act copy per batch.
- SWDGE scratch ring raised 16K->64K so a 2048-edge gather fits.
- Aggregation z = S^T @ (p * h1) is already in h1 space: no per-tile
  W1-apply / z transposes at finalize.
"""

import os
from contextlib import ExitStack

import numpy as np

# ----------------------------------------------------------------------------
# problem config (hardcoded per contest contract)
# ----------------------------------------------------------------------------
CFG = dict(
    N=50000,       # nodes
    IN=128,        # input feature dim
    HID=64,        # per-head hidden dim
    H1=8,          # layer-1 heads
    NCORES=8,
)

P = 128   # partitions / tile edge
GCH = 4   # edge groups per dma_gather
MCH = 4   # edge groups per L2 message-product chunk
SCRATCH = 65536  # SWDGE ring bytes/partition (4096 descriptors)


def _cdiv(a, b):
    return (a + b - 1) // b


# ----------------------------------------------------------------------------
# host-side sharding prep (pure layout work: sort, bucket, pad, pack indices)
# ----------------------------------------------------------------------------
def prep_edges(edge_index, cfg):
    """Partition self-loop-augmented edges by dst range across cores; within
    each 128-dst tile split by src half (int16 index limit) and pad each group
    to a multiple of 128 edges.  Tile counts are equalized across cores so a
    single SPMD instruction stream fits every core.
    """
    N, NC = cfg["N"], cfg["NCORES"]
    NPC = N // NC              # nodes per core
    NT = _cdiv(NPC, P)         # dst tiles per core
    HALF = _cdiv(N, 2)

    src = np.concatenate([edge_index[0].astype(np.int64), np.arange(N, dtype=np.int64)])
    dst = np.concatenate([edge_index[1].astype(np.int64), np.arange(N, dtype=np.int64)])
    order = np.argsort(dst, kind="stable")
    src, dst = src[order], dst[order]

    groups = [[None] * NT for _ in range(NC)]
    core_of = dst // NPC
    core_bounds = np.searchsorted(core_of, np.arange(NC + 1))
    for c in range(NC):
        s0, s1 = core_bounds[c], core_bounds[c + 1]
        d_loc = dst[s0:s1] - c * NPC
        tile_bounds = np.searchsorted(d_loc, np.arange(0, NT * P + 1, P))
        for t in range(NT):
            e0, e1 = s0 + tile_bounds[t], s0 + tile_bounds[t + 1]
            s_t = src[e0:e1]
            slot_t = (dst[e0:e1] - c * NPC - t * P).astype(np.int64)
            lo = s_t < HALF
            groups[c][t] = (s_t[lo], slot_t[lo], s_t[~lo] - HALF, slot_t[~lo])

    sched = []
    for t in range(NT):
        Lt = max(_cdiv(len(groups[c][t][0]), P) for c in range(NC))
        Ht = max(_cdiv(len(groups[c][t][2]), P) for c in range(NC))
        sched.append((Lt, Ht))

    total_groups = sum(l + h for l, h in sched)
    TI = total_groups * P          # total padded edges per core

    idx16 = np.zeros((NC, 16, TI // 16), dtype=np.int16)
    dstslot = np.full((NC, P, total_groups), -1.0, dtype=np.float32)

    for c in range(NC):
        off = 0
        for t in range(NT):
            lo_s, lo_k, hi_s, hi_k = groups[c][t]
            Lt, Ht = sched[t]
            for (ss, kk, ng) in ((lo_s, lo_k, Lt), (hi_s, hi_k, Ht)):
                n = ng * P
                if n == 0:
                    continue
                si = np.zeros(n, dtype=np.int64)
                si[: len(ss)] = ss
                ki = np.full(n, -1.0, dtype=np.float32)
                ki[: len(kk)] = kk
                idx16[c, :, off // 16: (off + n) // 16] = (
                    si.reshape(-1, 16).T.astype(np.int16)
                )
                g0 = off // P
                dstslot[c, :, g0: g0 + ng] = ki.reshape(-1, P).T
                off += n
        assert off == TI
    idx16 = np.tile(idx16, (1, 8, 1))
    return sched, idx16, dstslot, HALF


# ----------------------------------------------------------------------------
# device kernel
# ----------------------------------------------------------------------------
def build_kernel(cfg, sched, TI, HALF, profile=False):
    import concourse.bacc as bacc
    import concourse.mybir as mybir
    import concourse.tile as tile
    from concourse.masks import make_identity

    N, IN, HID, H1, NC = cfg["N"], cfg["IN"], cfg["HID"], cfg["H1"], cfg["NCORES"]
    NPC = N // NC
    NT = _cdiv(NPC, P)
    NPCP = NT * P                  # padded local rows
    OUT1 = H1 * HID
    TG = TI // P
    W2C = _cdiv(OUT1, P)           # W2 row chunks
    f32, bf16 = mybir.dt.float32, mybir.dt.bfloat16
    i16, i32 = mybir.dt.int16, mybir.dt.int32
    AX = mybir.AxisListType
    ALU = mybir.AluOpType
    ACTF = mybir.ActivationFunctionType
    RG = [list(range(NC))]

    nc = bacc.Bacc("TRN2", target_bir_lowering=False, debug=False,
                   num_devices=1 if profile else NC,
                   dynamic_dma_scratch_size=SCRATCH)

    # ---- I/O ----
    x_sl = nc.dram_tensor("x_slice", [NPC, IN], f32, kind="ExternalInput")
    W1_d = nc.dram_tensor("W1", [IN, OUT1], f32, kind="ExternalInput")
    as1_d = nc.dram_tensor("att_src1", [H1, HID], f32, kind="ExternalInput")
    ad1_d = nc.dram_tensor("att_dst1", [H1, HID], f32, kind="ExternalInput")
    b1_d = nc.dram_tensor("b1", [OUT1], f32, kind="ExternalInput")
    W2_d = nc.dram_tensor("W2", [OUT1, HID], f32, kind="ExternalInput")
    as2_d = nc.dram_tensor("att_src2", [1, HID], f32, kind="ExternalInput")
    ad2_d = nc.dram_tensor("att_dst2", [1, HID], f32, kind="ExternalInput")
    b2_d = nc.dram_tensor("b2", [HID], f32, kind="ExternalInput")
    fcw_d = nc.dram_tensor("fc_w", [HID, 1], f32, kind="ExternalInput")
    fcb_d = nc.dram_tensor("fc_b", [1], f32, kind="ExternalInput")
    idx_d = nc.dram_tensor("idx16", [P, TI // 16], i16, kind="ExternalInput")
    slot_d = nc.dram_tensor("dstslot", [P, TG], f32, kind="ExternalInput")
    out_d = nc.dram_tensor("out", [NPC, 1], f32, kind="ExternalOutput")

    # ---- internal DRAM ----
    xbf_in = nc.dram_tensor("xbf_in", [NPCP, IN], bf16)
    xbf = nc.dram_tensor("xbf", [N, IN], bf16, addr_space="Shared")
    h2e_in = nc.dram_tensor("h2e_in", [NPCP, P], bf16)
    h2e = nc.dram_tensor("h2e", [N, P], bf16, addr_space="Shared")
    ssum_in = nc.dram_tensor("ssum_in", [1, 1], f32)
    ssum = nc.dram_tensor("ssum", [1, 1], f32, addr_space="Shared")

    with tile.TileContext(nc) as tc, ExitStack() as ctx:
        const = ctx.enter_context(tc.tile_pool(name="const", bufs=1))
        sb = ctx.enter_context(tc.tile_pool(name="sb", bufs=2))
        sb1 = ctx.enter_context(tc.tile_pool(name="sb1", bufs=1))
        sb3 = ctx.enter_context(tc.tile_pool(name="sb3", bufs=3))
        psA = ctx.enter_context(tc.tile_pool(name="psA", bufs=1, space="PSUM"))
        psB = ctx.enter_context(tc.tile_pool(name="psB", bufs=2, space="PSUM"))
        psC = ctx.enter_context(tc.tile_pool(name="psC", bufs=1, space="PSUM"))

        # ================= constants / weights =================
        idbf = const.tile([P, P], bf16)
        make_identity(nc, idbf[:])
        iota_i = const.tile([P, P], i32)
        nc.gpsimd.iota(iota_i[:], pattern=[[1, P]], base=0,
                       channel_multiplier=0)
        ones_r = const.tile([1, P], f32)
        nc.vector.memset(ones_r[:], 1.0)
        ones_c = const.tile([P, 1], f32)
        nc.vector.memset(ones_c[:], 1.0)
        zpad = const.tile([P, P], bf16)
        nc.vector.memset(zpad[:], 0.0)

        idx16_sb = const.tile([P, TI // 16], i16)
        nc.sync.dma_start(idx16_sb[:], idx_d.ap())
        slot_sb = const.tile([P, TG], f32)
        nc.sync.dma_start(slot_sb[:], slot_d.ap())
        slot_bf = const.tile([P, TG], bf16)
        nc.vector.tensor_copy(slot_bf[:], slot_sb[:])
        iota_bf = const.tile([P, P], bf16)
        nc.vector.tensor_copy(iota_bf[:], iota_i[:])

        w1f = const.tile([P, OUT1], f32)
        nc.sync.dma_start(w1f[:], W1_d.ap())
        w1b = const.tile([P, OUT1], bf16)
        nc.vector.tensor_copy(w1b[:], w1f[:])
        w2b = const.tile([P, W2C, HID], bf16)
        w2f_t = sb.tile([P, W2C, HID], f32, tag="tmpw")
        nc.sync.dma_start(
            w2f_t[:], W2_d.ap().rearrange("(c p) n -> p c n", p=P))
        nc.vector.tensor_copy(w2b[:], w2f_t[:])

        def bcast_row(dram_ap, width, name):
            row = sb.tile([1, width], f32, tag="bcrow")
            nc.sync.dma_start(row[:], dram_ap)
            pt = psB.tile([P, width], f32, tag="h1")
            nc.tensor.matmul(pt[:], lhsT=ones_r[:], rhs=row[:], start=True,
                             stop=True)
            out = const.tile([P, width], f32, tag=name)
            nc.scalar.copy(out[:], pt[:])
            return out

        att1s_bc = bcast_row(
            as1_d.ap().rearrange("(o h) d -> o (h d)", o=1), OUT1, "a1s")
        att1d_bc = bcast_row(
            ad1_d.ap().rearrange("(o h) d -> o (h d)", o=1), OUT1, "a1d")
        att2s_bc = bcast_row(as2_d.ap(), HID, "a2s")
        att2d_bc = bcast_row(ad2_d.ap(), HID, "a2d")
        b1_bc = bcast_row(b1_d.ap()[None, :], OUT1, "b1")
        b2_bc = bcast_row(b2_d.ap()[None, :], HID, "b2")
        fcb_bc = bcast_row(fcb_d.ap()[None, :], 1, "fcb")

        def fold_att(att_bc, name):
            tmp = sb.tile([P, OUT1], f32, tag="tmpw2")
            nc.vector.tensor_tensor(tmp[:], w1f[:], att_bc[:], op=ALU.mult)
            red = sb.tile([P, H1], f32, tag="tmpw3")
            nc.vector.tensor_reduce(
                red[:], tmp[:].rearrange("p (h d) -> p h d", h=H1),
                axis=AX.X, op=ALU.add)
            out = const.tile([P, H1], bf16, tag=name)
            nc.vector.tensor_copy(out[:], red[:])
            return out

        wsrc = fold_att(att1s_bc, "wsrc")
        wdst = fold_att(att1d_bc, "wdst")

        fcw_f = sb.tile([HID, 1], f32, tag="tmpw4")
        nc.sync.dma_start(fcw_f[:], fcw_d.ap())
        fcw_sb = const.tile([HID, 1], bf16)
        nc.vector.tensor_copy(fcw_sb[:], fcw_f[:])

        # ================= phase 1: x -> bf16, AllGather =================
        for k in range(NT):
            r0, r1 = k * P, min(k * P + P, NPC)
            xf = sb3.tile([P, IN], f32, tag="xcast")
            nc.sync.dma_start(xf[: r1 - r0], x_sl.ap()[r0:r1, :])
            xb = sb3.tile([P, IN], bf16, tag="xcastb")
            nc.vector.tensor_copy(xb[: r1 - r0], xf[: r1 - r0])
            nc.sync.dma_start(xbf_in.ap()[r0:r1, :], xb[: r1 - r0])
        if NPCP > NPC:  # zero local pad rows (read by t=NT-1 transpose load)
            nc.sync.dma_start(xbf_in.ap()[NPC:NPCP, :],
                              zpad[: NPCP - NPC, :IN])
        if profile:
            nc.sync.dma_start(xbf.ap()[0:NPC, :], xbf_in.ap()[0:NPC, :])
        else:
            nc.gpsimd.collective_compute(
                "AllGather", ALU.bypass, replica_groups=RG,
                ins=[xbf_in.ap()[0:NPC, :].opt()],
                outs=[xbf.ap()[0:N, :].opt()])

        logits = const.tile([P, NT], f32, tag="logits")
        nc.vector.memset(logits[:], -1e30)

        # ================= shared edge-phase machinery =====================
        def edge_phase(layer):
            L1 = layer == 1
            table = xbf if L1 else h2e
            local = xbf_in if L1 else h2e_in
            NH = H1 if L1 else 1        # heads
            lo_ap = table.ap()[0:HALF, :]
            hi_ap = table.ap()[HALF:N, :]
            goff = 0
            for t in range(NT):
                Lt, Ht = sched[t]
                Kt = Lt + Ht
                gbase = t * P
                rows_t = min(NPC - t * P, P)

                # --- node-side block (local slice; transposed load) ---
                ndT = sb.tile([P, P if not L1 else IN], bf16, tag="ndT")
                nc.sync.dma_start(ndT[:], local.ap()[gbase:gbase + P, :],
                                  transpose=True)
                adn_p = psB.tile([P, NH], f32, tag="adn")
                if L1:
                    nc.tensor.matmul(adn_p[:], lhsT=ndT[:], rhs=wdst[:],
                                     start=True, stop=True)
                else:
                    nc.tensor.matmul(adn_p[:], lhsT=ndT[:],
                                     rhs=idbf[:, HID + 1:HID + 2],
                                     start=True, stop=True)
                adn = sb.tile([P, NH], bf16, tag="adn_s")
                nc.scalar.copy(adn[:], adn_p[:])

                # --- single gather per chunk ---
                # L1: transposed [feat, edge] (h1/asrc via PE matmuls)
                # L2: direct [edge, feat] (asrc2 is packed col HID)
                off16 = goff * P // 16
                if L1:
                    gt = sb.tile([P, Kt * P], bf16, tag="gath")
                else:
                    X2 = sb.tile([P, Kt, P], bf16, tag="gath")
                for g0, gn, half_ap in (
                        [(q, min(GCH, Lt - q), lo_ap)
                         for q in range(0, Lt, GCH)]
                        + [(Lt + q, min(GCH, Ht - q), hi_ap)
                           for q in range(0, Ht, GCH)]):
                    n = gn * P
                    idxs = idx16_sb[:, off16 + g0 * P // 16:
                                    off16 + (g0 * P + n) // 16]
                    if L1:
                        nc.gpsimd.dma_gather(
                            gt[:, None, g0 * P: g0 * P + n], half_ap, idxs,
                            n, n, P, transpose=True)
                    else:
                        nc.gpsimd.dma_gather(
                            X2[:, g0: g0 + gn, :], half_ap, idxs,
                            n, n, P, transpose=False)

                # --- selection matrices for all Kt groups ---
                S_all = sb.tile([P, Kt, P], bf16, tag="S")
                nc.vector.tensor_tensor(
                    S_all[:],
                    iota_bf[:, None, :].to_broadcast([P, Kt, P]),
                    slot_bf[:, goff:goff + Kt, None].to_broadcast([P, Kt, P]),
                    op=ALU.is_equal)
                goff += Kt

                # --- S^T for all groups: PE transposes batched 8 per
                #     PSUM bank, one Act copy per batch ---
                st_all = sb1.tile([P, Kt, P], bf16, tag="st")
                for b0 in range(0, Kt, 8):
                    bk = min(8, Kt - b0)
                    stb = psC.tile([P, 8, P], bf16, tag="stb")
                    for j in range(b0, b0 + bk):
                        nc.tensor.transpose(stb[:, j - b0, :], S_all[:, j, :],
                                            idbf[:])
                    nc.scalar.copy(st_all[:, b0:b0 + bk, :], stb[:, 0:bk, :])

                # --- pass 1: attention logits for the whole tile ---
                ae_p = psA.tile([P, Kt * NH], f32, tag="ae")
                if L1:
                    for j in range(Kt):
                        nc.tensor.matmul(ae_p[:, j * NH:(j + 1) * NH],
                                         lhsT=gt[:, j * P:(j + 1) * P],
                                         rhs=wsrc[:],
                                         start=(j == 0), stop=False)
                        nc.tensor.matmul(ae_p[:, j * NH:(j + 1) * NH],
                                         lhsT=st_all[:, j, :], rhs=adn[:],
                                         start=False, stop=(j == Kt - 1))
                    lr = sb.tile([P, Kt * NH], f32, tag="lr")
                    nc.scalar.activation(lr[:], ae_p[:], ACTF.Prelu,
                                         alpha=0.2)
                else:
                    for j in range(Kt):
                        nc.tensor.matmul(ae_p[:, j:j + 1],
                                         lhsT=st_all[:, j, :],
                                         rhs=adn[:, 0:1],
                                         start=(j == 0), stop=(j == Kt - 1))
                    esum = sb.tile([P, Kt], f32, tag="esum")
                    nc.vector.tensor_tensor(
                        esum[:, :, None], ae_p[:, :, None],
                        X2[:, :, HID:HID + 1], op=ALU.add)
                    lr = sb.tile([P, Kt], f32, tag="lr")
                    nc.scalar.activation(lr[:], esum[:], ACTF.Prelu,
                                         alpha=0.2)
                p_all = sb.tile([P, Kt, NH], bf16, tag="p")
                nc.scalar.activation(
                    p_all[:].rearrange("p k h -> p (k h)"), lr[:], ACTF.Exp)

                # --- pass 2: messages + segment sums ---
                if L1:
                    z_p = psA.tile([P, OUT1], f32, tag="z")
                    s_p = psA.tile([P, NH], f32, tag="s")
                    for j in range(Kt):
                        h1_p = psB.tile([P, OUT1], f32, tag="h1")
                        nc.tensor.matmul(h1_p[:],
                                         lhsT=gt[:, j * P:(j + 1) * P],
                                         rhs=w1b[:], start=True, stop=True)
                        M_c = sb3.tile([P, NH, HID], bf16, tag="M")
                        nc.vector.tensor_tensor(
                            M_c[:],
                            h1_p[:].rearrange("p (h f) -> p h f", h=NH),
                            p_all[:, j, :, None].to_broadcast([P, NH, HID]),
                            op=ALU.mult)
                        nc.tensor.matmul(
                            z_p[:], lhsT=S_all[:, j, :],
                            rhs=M_c[:].rearrange("p h f -> p (h f)"),
                            start=(j == 0), stop=(j == Kt - 1))
                        nc.tensor.matmul(
                            s_p[:], lhsT=S_all[:, j, :], rhs=p_all[:, j, :],
                            start=(j == 0), stop=(j == Kt - 1))
                else:
                    z_p = psA.tile([P, HID + 1], f32, tag="z")
                    for m0 in range(0, Kt, MCH):
                        m1 = min(m0 + MCH, Kt)
                        mk = m1 - m0
                        M_c = sb3.tile([P, MCH, HID + 1], bf16, tag="M2")
                        nc.vector.tensor_tensor(
                            M_c[:, 0:mk, 0:HID],
                            X2[:, m0:m1, 0:HID],
                            p_all[:, m0:m1, :].to_broadcast([P, mk, HID]),
                            op=ALU.mult)
                        nc.vector.tensor_copy(M_c[:, 0:mk, HID:HID + 1],
                                              p_all[:, m0:m1, :])
                        for j in range(m0, m1):
                            nc.tensor.matmul(
                                z_p[:], lhsT=S_all[:, j, :],
                                rhs=M_c[:, j - m0, :],
                                start=(j == 0), stop=(j == Kt - 1))

                # ---------------- finalize dst tile ----------------
                if L1:
                    s_eps = sb.tile([P, NH], f32, tag="seps")
                    nc.vector.tensor_scalar(s_eps[:], s_p[:], 1e-16, None,
                                            op0=ALU.add)
                    s_inv = sb.tile([P, NH], f32, tag="sinv")
                    nc.vector.reciprocal(s_inv[:], s_eps[:])
                    y = sb.tile([P, OUT1], f32, tag="y")
                    nc.vector.tensor_tensor(
                        y[:].rearrange("p (h f) -> p h f", h=H1),
                        z_p[:].rearrange("p (h f) -> p h f", h=H1),
                        s_inv[:, :, None].to_broadcast([P, H1, HID]),
                        op=ALU.mult)
                    nc.vector.tensor_tensor(y[:], y[:], b1_bc[:], op=ALU.add)
                    WE = OUT1
                else:
                    s_eps = sb.tile([P, 1], f32, tag="seps")
                    nc.vector.tensor_scalar(s_eps[:], z_p[:, HID:HID + 1],
                                            1e-16, None, op0=ALU.add)
                    s_inv = sb.tile([P, 1], f32, tag="sinv")
                    nc.vector.reciprocal(s_inv[:], s_eps[:])
                    y = sb.tile([P, HID], f32, tag="y")
                    nc.vector.tensor_scalar(
                        y[:], z_p[:, 0:HID], s_inv[:], None,
                        op0=ALU.mult)
                    nc.vector.tensor_tensor(y[:], y[:], b2_bc[:], op=ALU.add)
                    WE = HID
                # elu(y) = relu(y) + exp(min(y,0)) - 1
                t0 = sb.tile([P, WE], f32, tag="elu0")
                nc.vector.tensor_scalar_min(t0[:], y[:], 0.0)
                ex = sb.tile([P, WE], f32, tag="elu1")
                nc.scalar.activation(ex[:], t0[:], ACTF.Exp)
                ry = sb.tile([P, WE], f32, tag="elu2")
                nc.scalar.activation(ry[:], y[:], ACTF.Relu)
                x2 = sb.tile([P, WE], bf16, tag="x2")
                nc.vector.scalar_tensor_tensor(
                    x2[:], in0=ex[:], scalar=-1.0, in1=ry[:],
                    op0=ALU.add, op1=ALU.add)

                if L1:
                    # h2 = x2 @ W2 via 4 transposed chunks
                    xtb = psC.tile([P, 8, P], bf16, tag="stb")
                    for cix in range(W2C):
                        nc.tensor.transpose(
                            xtb[:, cix, :], x2[:, cix * P:(cix + 1) * P],
                            idbf[:])
                    xts = sb3.tile([P, W2C, P], bf16, tag="xts")
                    nc.scalar.copy(xts[:], xtb[:, 0:W2C, :])
                    h2_p = psA.tile([P, HID], f32, tag="s")
                    for cix in range(W2C):
                        nc.tensor.matmul(h2_p[:], lhsT=xts[:, cix, :],
                                         rhs=w2b[:, cix, :],
                                         start=(cix == 0),
                                         stop=(cix == W2C - 1))
                    h2e_sb = sb.tile([P, P], bf16, tag="h2e")
                    nc.vector.memset(h2e_sb[:], 0.0)
                    nc.scalar.copy(h2e_sb[:, 0:HID], h2_p[:])
                    tmp = sb.tile([P, HID], f32, tag="atmp")
                    ared = sb.tile([P, 1], f32, tag="ared")
                    nc.vector.tensor_tensor(tmp[:], h2_p[:], att2s_bc[:],
                                            op=ALU.mult)
                    nc.vector.tensor_reduce(ared[:], tmp[:], axis=AX.X,
                                            op=ALU.add)
                    nc.vector.tensor_copy(h2e_sb[:, HID:HID + 1], ared[:])
                    ared2 = sb.tile([P, 1], f32, tag="ared2")
                    nc.vector.tensor_tensor(tmp[:], h2_p[:], att2d_bc[:],
                                            op=ALU.mult)
                    nc.vector.tensor_reduce(ared2[:], tmp[:], axis=AX.X,
                                            op=ALU.add)
                    nc.vector.tensor_copy(h2e_sb[:, HID + 1:HID + 2],
                                          ared2[:])
                    nc.sync.dma_start(
                        h2e_in.ap()[gbase:gbase + rows_t, :],
                        h2e_sb[0:rows_t, :])
                else:
                    x2t_p = psB.tile([HID, P], bf16, tag="adn")
                    nc.tensor.transpose(x2t_p[:], x2[:, 0:HID], idbf[:])
                    x2t = sb3.tile([HID, P], bf16, tag="x2t")
                    nc.scalar.copy(x2t[:], x2t_p[:])
                    lg_p = psB.tile([P, 1], f32, tag="adn")
                    nc.tensor.matmul(lg_p[:], lhsT=x2t[:], rhs=fcw_sb[:],
                                     start=True, stop=True)
                    nc.scalar.activation(logits[0:rows_t, t:t + 1],
                                         lg_p[0:rows_t, :], ACTF.Identity,
                                         bias=fcb_bc[0:rows_t, :])

        # ================= layer 1 =================
        edge_phase(1)
        if NPCP > NPC:
            nc.sync.dma_start(h2e_in.ap()[NPC:NPCP, :], zpad[: NPCP - NPC, :])
        if profile:
            nc.sync.dma_start(h2e.ap()[0:NPC, :], h2e_in.ap()[0:NPC, :])
        else:
            nc.gpsimd.collective_compute(
                "AllGather", ALU.bypass, replica_groups=RG,
                ins=[h2e_in.ap()[0:NPC, :].opt()],
                outs=[h2e.ap()[0:N, :].opt()])

        # ================= layer 2 =================
        edge_phase(2)

        # ================= softmax over all nodes =================
        ex_all = sb.tile([P, NT], f32, tag="exall")
        nc.scalar.activation(ex_all[:], logits[:], ACTF.Exp)
        part = sb.tile([P, 1], f32, tag="part")
        nc.vector.tensor_reduce(part[:], ex_all[:], axis=AX.X, op=ALU.add)
        tot_p = psB.tile([1, 1], f32, tag="adn")
        nc.tensor.matmul(tot_p[:], lhsT=part[:], rhs=ones_c[:], start=True,
                         stop=True)
        tot_sb = sb.tile([1, 1], f32, tag="tot")
        nc.scalar.copy(tot_sb[:], tot_p[:])
        nc.sync.dma_start(ssum_in.ap(), tot_sb[:])
        if profile:
            nc.sync.dma_start(ssum.ap(), ssum_in.ap())
        else:
            nc.gpsimd.collective_compute(
                "AllReduce", ALU.add, replica_groups=RG,
                ins=[ssum_in.ap().opt()], outs=[ssum.ap().opt()])
        gsum = sb.tile([1, 1], f32, tag="gsum")
        nc.sync.dma_start(gsum[:], ssum.ap())
        ginv = sb.tile([1, 1], f32, tag="ginv")
        nc.vector.reciprocal(ginv[:], gsum[:])
        ginv_p = psB.tile([P, 1], f32, tag="adn")
        nc.tensor.matmul(ginv_p[:], lhsT=ones_r[:], rhs=ginv[:], start=True,
                         stop=True)
        ginv_bc = sb.tile([P, 1], f32, tag="ginvbc")
        nc.scalar.copy(ginv_bc[:], ginv_p[:])
        res = sb.tile([P, NT], f32, tag="res")
        nc.vector.tensor_scalar(res[:], ex_all[:], ginv_bc[:], None,
                                op0=ALU.mult)
        full_t = NPC // P
        nc.sync.dma_start(
            out_d.ap()[0:full_t * P, :].rearrange("(t p) o -> p (t o)", p=P),
            res[:, 0:full_t])
        if NPC % P:
            nc.sync.dma_start(out_d.ap()[full_t * P: NPC, :],
                              res[0: NPC % P, full_t:full_t + 1])

    nc.compile()
    return nc


# ----------------------------------------------------------------------------
# entry point
# ----------------------------------------------------------------------------
def build_in_maps(inputs, cfg):
    sched, idx16, dstslot, HALF = prep_edges(
        np.asarray(inputs["edge_index"]), cfg)
    x = np.asarray(inputs["x"], dtype=np.float32)
    NC = cfg["NCORES"]
    NPC = cfg["N"] // NC
    common = {k: np.ascontiguousarray(np.asarray(inputs[k], np.float32))
              for k in ("W1", "att_src1", "att_dst1", "b1", "W2", "att_src2",
                        "att_dst2", "b2", "fc_w", "fc_b")}
    in_maps = []
    for c in range(NC):
        m = dict(common)
        m["x_slice"] = np.ascontiguousarray(x[c * NPC:(c + 1) * NPC])
        m["idx16"] = np.ascontiguousarray(idx16[c])
        m["dstslot"] = np.ascontiguousarray(dstslot[c])
        in_maps.append(m)
    TI = dstslot.shape[2] * P
    return in_maps, sched, TI, HALF


def kernel(**inputs) -> np.ndarray:
    from concourse import bass_utils

    cfg = dict(CFG)
    in_maps, sched, TI, HALF = build_in_maps(inputs, cfg)
    nc = build_kernel(cfg, sched, TI, HALF)
    res = bass_utils.run_bass_kernel_spmd(
        nc, in_maps, core_ids=list(range(cfg["NCORES"])),
        trace=bool(int(os.environ.get("GAT_TRACE", "0"))))
    kernel.last_results = res
    out = np.concatenate([r["out"] for r in res.results], axis=0)
    return out.astype(np.float32)
